# revision 4
# baseline (speedup 1.0000x reference)
"""ChildSum TreeLSTM cell on 8 Trainium2 NeuronCores (Bass/Tile, SPMD).

Sharding: nodes split evenly (2048/core); each core's children (contiguous,
since seg_ids is sorted) are re-laid out host-side into a window-aligned
padded layout: 16 node-windows of 128 nodes per core, each window's children
padded to KMAX slots of 128 rows.  Segment sums are per-window one-hot
matmuls accumulated in PSUM; the one-hot masks are precomputed host-side in
both orientations (child-major and node-major) and streamed in bf16.

v3: all streams bf16 (x, prev_c, masks, Wc, outputs) or fp8 (prev_h^T, Wuf
scaled x16, unscaled inside the sigmoid); the Wuf matmul runs in fp8
DoubleRow perf mode (2 k-tiles/pass); elementwise work is spread across
DVE / ACT / Pool so the PE stays the only near-saturated engine; all DMA
layouts are partition-major so each descriptor moves 1-5 KiB contiguous.

Per-core device program (per window w of 128 nodes):
  f_inp[w] = 16*(x_w @ Wwf.T + bwf+buf)            (bf16, PSUM->DVE add)
  per slot s (128 children):
    fhg   = prevh_s @ (16*Wuf).T  (fp8 DoubleRow)  + S_nc @ f_inp[w]
    f_jk  = sigmoid(fhg/16)                         (ACT, ->bf16)
  t      = f_jk * prevc                             (DVE, one op per window)
  fc[w]  = sum_s S_cn.T @ t_s                       (PSUM accum)
  htT[w] = sum_s prevh_s.T @ S_cn_s                 (PSUM accum, transposed)
  big = [x_w; h_tilde] @ Wc.T (bf16) ; z = big+bc (DVE) ; gates (ACT)
  c = zi*zu + fc (Pool) ; h = zo*tanh(c) (ACT+Pool) ; store bf16
"""

import numpy as np
import ml_dtypes

import concourse.bass as bass
import concourse.bacc as bacc
import concourse.mybir as mybir
from concourse import tile
from concourse.bass_utils import run_bass_kernel_spmd

BF16 = ml_dtypes.bfloat16
FP8 = ml_dtypes.float8_e4m3
F32 = mybir.dt.float32
BF = mybir.dt.bfloat16
F8 = mybir.dt.float8e4

N, E, D, H = 16384, 65536, 512, 512
NCORES = 8
NL = N // NCORES            # 2048 local nodes
NW = NL // 128              # 16 windows
H3 = 3 * H
WSC = 16.0                  # Wuf/Wwf fp8 scale, undone in the f_jk sigmoid

AF = mybir.ActivationFunctionType
ALU = mybir.AluOpType
DR = mybir.MatmulPerfMode.DoubleRow


# ---------------------------------------------------------------------------
# Host-side shard planning and per-core data layout
# ---------------------------------------------------------------------------
def _plan(seg):
    win_edges = np.arange(0, N + 1, 128)
    wchild = np.searchsorted(seg, win_edges)
    kmax = int(np.max(np.ceil(np.diff(wchild) / 128.0)))
    return wchild, max(kmax, 1)


def _prep_core(inputs, core, wchild, kmax):
    seg = inputs["seg_ids"]
    x, prev_c, prev_h = inputs["x"], inputs["prev_c"], inputs["prev_h"]
    g0 = core * NL
    S1 = kmax * 128

    ph_pad = np.zeros((NW, S1, H), np.float32)
    pc_pad = np.zeros((NW, S1, H), np.float32)
    rel = np.full((NW, S1), -1, np.int32)
    for w in range(NW):
        gw = core * NW + w
        ws, we = int(wchild[gw]), int(wchild[gw + 1])
        n = we - ws
        ph_pad[w, :n] = prev_h[ws:we]
        pc_pad[w, :n] = prev_c[ws:we]
        rel[w, :n] = seg[ws:we] - (g0 + 128 * w)

    onehot = (rel[:, :, None] == np.arange(128)[None, None, :])  # [NW,S1,128]
    xc = x[g0 : g0 + NL].reshape(NW, 128, 4, 128)                # [w,j,q,p]

    return {
        "xTl": np.ascontiguousarray(xc.transpose(0, 3, 2, 1)).astype(BF16),
        "phT8": np.ascontiguousarray(
            ph_pad.reshape(NW, S1, 4, 128).transpose(0, 3, 2, 1)
        ).astype(FP8),                                           # [w,p,q,s]
        "phn": np.ascontiguousarray(
            ph_pad.reshape(NW, kmax, 128, H).transpose(0, 2, 1, 3)
        ).astype(BF16),                                          # [w,r,k,h]
        "pcn": np.ascontiguousarray(
            pc_pad.reshape(NW, kmax, 128, H).transpose(0, 2, 1, 3)
        ).astype(BF16),                                          # [w,r,k,h]
        "s16t": np.ascontiguousarray(
            onehot.reshape(NW, kmax, 128, 128).transpose(0, 2, 1, 3)
            .reshape(NW, 128, S1)
        ).astype(BF16),                                          # [w,r,k*128+j]
        "sncm": np.ascontiguousarray(onehot.transpose(0, 2, 1)).astype(BF16),
    }


def _prep_shared(inputs):
    Wc, bc = inputs["Wc"], inputs["bc"]
    Wwf, bwf = inputs["Wwf"], inputs["bwf"]
    Wuf, buf = inputs["Wuf"], inputs["buf"]
    ones = np.ones((128, 1), np.float32)
    return {
        "wwf": np.ascontiguousarray(
            (WSC * Wwf.T).reshape(4, 128, H).transpose(1, 0, 2)
        ).astype(BF16),                                          # [p,q,h]
        "wuf8": np.ascontiguousarray(
            (WSC * Wuf.T).reshape(4, 128, H).transpose(1, 0, 2)
        ).astype(FP8),                                           # [p,q,h]
        "wcb": np.ascontiguousarray(
            Wc.T.reshape(8, 128, H3).transpose(1, 0, 2)
        ).astype(BF16),                                          # [p,kc,z]
        "b1": (ones * (WSC * (bwf + buf))[None, :]).astype(np.float32),
        "bcb": (ones * bc[None, :]).astype(np.float32),
    }


# ---------------------------------------------------------------------------
# Device program (identical for all cores; per-core data differs)
# ---------------------------------------------------------------------------
def _build_program(kmax, repeat=1):
    """repeat>1 wraps the whole body in a hardware loop (timing harness only)."""
    S1 = kmax * 128

    nc = bacc.Bacc(None, target_bir_lowering=False)
    d_xT = nc.dram_tensor("xTl", [NW, 128, 4, 128], BF, kind="ExternalInput")
    d_ph8 = nc.dram_tensor("phT8", [NW, 128, 4, S1], F8, kind="ExternalInput")
    d_phn = nc.dram_tensor("phn", [NW, 128, kmax, H], BF, kind="ExternalInput")
    d_pc = nc.dram_tensor("pcn", [NW, 128, kmax, H], BF, kind="ExternalInput")
    d_st = nc.dram_tensor("s16t", [NW, 128, S1], BF, kind="ExternalInput")
    d_sn = nc.dram_tensor("sncm", [NW, 128, S1], BF, kind="ExternalInput")
    d_wwf = nc.dram_tensor("wwf", [128, 4, H], BF, kind="ExternalInput")
    d_wuf = nc.dram_tensor("wuf8", [128, 4, H], F8, kind="ExternalInput")
    d_wc = nc.dram_tensor("wcb", [128, 8, H3], BF, kind="ExternalInput")
    d_b1 = nc.dram_tensor("b1", [128, H], F32, kind="ExternalInput")
    d_bcb = nc.dram_tensor("bcb", [128, H3], F32, kind="ExternalInput")
    d_c = nc.dram_tensor("c_out", [NL, H], BF, kind="ExternalOutput")
    d_h = nc.dram_tensor("h_out", [NL, H], BF, kind="ExternalOutput")

    import contextlib

    with tile.TileContext(nc) as tc:
        with (
            tc.tile_pool(name="const", bufs=1) as cpool,
            tc.tile_pool(name="stream", bufs=3) as spool,
            tc.tile_pool(name="work", bufs=3) as wpool,
            tc.tile_pool(name="gates", bufs=3) as gpool,
            tc.tile_pool(name="pfhg", bufs=2, space="PSUM") as pfhg,
            tc.tile_pool(name="phtT", bufs=2, space="PSUM") as phtT,
            tc.tile_pool(name="pfc", bufs=2, space="PSUM") as pfc,
            tc.tile_pool(name="pbig", bufs=2, space="PSUM") as pbig,
            tc.For_i(0, repeat, 1) if repeat > 1 else contextlib.nullcontext(),
        ):
            # ---- resident constants -------------------------------------
            b1 = cpool.tile([128, H], F32)
            nc.sync.dma_start(b1[:], d_b1[:])
            wwf = cpool.tile([128, 4, H], BF)
            nc.sync.dma_start(wwf[:], d_wwf[:])
            wuf = cpool.tile([128, 4, H], F8)
            nc.sync.dma_start(wuf[:], d_wuf[:])
            wc = cpool.tile([128, 8, H3], BF)
            bcb = cpool.tile([128, H3], F32)
            finp = cpool.tile([128, NW, H], BF)

            for w in range(NW):
                wsl = slice(128 * w, 128 * (w + 1))
                # f_inp for this window (bf16, scaled, bias folded in)
                xt = spool.tile([128, 4, 128], BF, tag="xt")
                nc.sync.dma_start(xt[:], d_xT[w])
                fp = pfhg.tile([128, H], F32, tag="fhg")
                for q in range(4):
                    nc.tensor.matmul(
                        fp[:], xt[:, q, :], wwf[:, q, :],
                        start=(q == 0), stop=(q == 3),
                    )
                nc.vector.tensor_tensor(finp[:, w, :], fp[:], b1[:], op=ALU.add)

                # window streams
                pht = spool.tile([128, 4, S1], F8, tag="pht")
                nc.sync.dma_start(pht[:], d_ph8[w])
                phn = spool.tile([128, kmax, H], BF, tag="phn")
                nc.sync.dma_start(phn[:], d_phn[w])
                pc = spool.tile([128, kmax, H], BF, tag="pc")
                nc.sync.dma_start(pc[:], d_pc[w])
                st = spool.tile([128, S1], BF, tag="st")
                nc.sync.dma_start(st[:], d_st[w])
                sn = spool.tile([128, S1], BF, tag="sn")
                nc.sync.dma_start(sn[:], d_sn[w])
                if w == 0:
                    # big Wc/bc loads deferred here: needed only at window 0's
                    # tail.  Split by z-chunk so the zc=0 slice lands first.
                    for zc in range(3):
                        zsl = slice(H * zc, H * (zc + 1))
                        nc.sync.dma_start(wc[:, :, zsl], d_wc[:, :, zsl])
                    nc.sync.dma_start(bcb[:], d_bcb[:])

                # f_jk per slot: fp8 DoubleRow Wuf matmul + one-hot gather
                fjk = wpool.tile([128, kmax, H], BF, tag="fjk")
                for k in range(kmax):
                    ksl = slice(128 * k, 128 * (k + 1))
                    fhg = pfhg.tile([128, H], F32, tag="fhg")
                    nc.tensor.matmul(
                        fhg[:], pht[:, 0:2, ksl], wuf[:, 0:2, :],
                        start=True, stop=False, perf_mode=DR,
                    )
                    nc.tensor.matmul(
                        fhg[:], pht[:, 2:4, ksl], wuf[:, 2:4, :],
                        start=False, stop=False, perf_mode=DR,
                    )
                    nc.tensor.matmul(
                        fhg[:], sn[:, ksl], finp[:, w, :],
                        start=False, stop=True,
                    )
                    nc.scalar.activation(
                        fjk[:, k, :], fhg[:], AF.Sigmoid, scale=1.0 / WSC
                    )

                t = wpool.tile([128, kmax, H], BF, tag="t")
                nc.vector.tensor_tensor(t[:], fjk[:], pc[:], op=ALU.mult)

                fcp = pfc.tile([128, H], F32, tag="fc")
                for k in range(kmax):
                    nc.tensor.matmul(
                        fcp[:], st[:, 128 * k : 128 * (k + 1)], t[:, k, :],
                        start=(k == 0), stop=(k == kmax - 1),
                    )
                # h_tilde^T: q outer so each psum slice runs one accumulation
                # group at a time.
                htp = phtT.tile([128, H], F32, tag="htT")
                for q in range(4):
                    for k in range(kmax):
                        nc.tensor.matmul(
                            htp[:, 128 * q : 128 * (q + 1)],
                            phn[:, k, 128 * q : 128 * (q + 1)],
                            st[:, 128 * k : 128 * (k + 1)],
                            start=(k == 0), stop=(k == kmax - 1),
                        )

                # window tail: drains + big matmul + gates
                fcs = gpool.tile([128, H], BF, tag="fcs")
                nc.scalar.copy(fcs[:], fcp[:])
                hts = gpool.tile([128, H], BF, tag="hts")
                nc.scalar.copy(hts[:], htp[:])

                zt = gpool.tile([128, 3, H], BF, tag="z")
                for zc in range(3):
                    bp = pbig.tile([128, H], F32, tag="big")
                    for kc in range(8):
                        if kc < 4:
                            lhsT = xt[:, kc, :]
                        else:
                            lhsT = hts[:, 128 * (kc - 4) : 128 * (kc - 3)]
                        nc.tensor.matmul(
                            bp[:], lhsT, wc[:, kc, H * zc : H * (zc + 1)],
                            start=(kc == 0), stop=(kc == 7),
                        )
                    nc.vector.tensor_tensor(
                        zt[:, zc, :], bp[:], bcb[:, H * zc : H * (zc + 1)],
                        op=ALU.add,
                    )
                nc.scalar.activation(zt[:, 0:2, :], zt[:, 0:2, :], AF.Sigmoid)
                nc.scalar.activation(zt[:, 2, :], zt[:, 2, :], AF.Tanh)
                ct = gpool.tile([128, H], BF, tag="ct")
                nc.vector.tensor_tensor(ct[:], zt[:, 0, :], zt[:, 2, :], op=ALU.mult)
                nc.vector.tensor_tensor(ct[:], ct[:], fcs[:], op=ALU.add)
                tct = gpool.tile([128, H], BF, tag="tct")
                nc.scalar.activation(tct[:], ct[:], AF.Tanh)
                ho = gpool.tile([128, H], BF, tag="ho")
                nc.vector.tensor_tensor(ho[:], zt[:, 1, :], tct[:], op=ALU.mult)
                nc.sync.dma_start(d_c[wsl, :], ct[:])
                nc.sync.dma_start(d_h[wsl, :], ho[:])

    nc.compile()
    return nc


# ---------------------------------------------------------------------------
# Entry point
# ---------------------------------------------------------------------------
def kernel(**inputs):
    inputs = {k: np.asarray(v) for k, v in inputs.items()}
    seg = inputs["seg_ids"]
    assert seg.shape == (E,) and np.all(np.diff(seg) >= 0)

    wchild, kmax = _plan(seg)
    shared = _prep_shared(inputs)
    in_maps = []
    for core in range(NCORES):
        m = dict(shared)
        m.update(_prep_core(inputs, core, wchild, kmax))
        in_maps.append(m)

    nc = _build_program(kmax)
    res = run_bass_kernel_spmd(nc, in_maps, list(range(NCORES)))

    c = np.concatenate([res.results[i]["c_out"] for i in range(NCORES)], axis=0)
    h = np.concatenate([res.results[i]["h_out"] for i in range(NCORES)], axis=0)
    return (c.astype(np.float32), h.astype(np.float32))
